# revision 1
# baseline (speedup 1.0000x reference)
"""Trainium2 Bass kernel for nn_DoubleAttentionPairBlock.

Reference computation (B=2, N=384, C=128, D=32, H=128, M=N*N):
    P  = edge_embed.reshape(B, M, C)
    fP = P @ Wg + bg ; gP = P @ Wv + bv
    weights = softmax(fP, axis=1)                 # over M
    V  = einsum('bmd,bme->bde', weights, gP)      # [B, D, D]
    scores = softmax(P @ Wd + bd, axis=2)         # over D
    O  = (scores @ V) @ Wo + bo
    out = edge_embed + O * (1 + scale) + shift    # FiLM from r3_t

Single-launch device strategy (8 cores, 4 shards per batch):
  Phase 1 per shard: psum-accumulate ZS[d, 0:32] = sum_m exp(fP)[m,d]*(P@Wv)[m,e]
     and ZS[d, 32] = sum_m exp(fP)[m,d]  (bg cancels in softmax over m);
     keep edge and exp(P@Wd) resident in SBUF.
  AllReduce ZS within each batch's 4-core group.
  On-device fold (host precomputes the tiny per-batch constants):
     Maug[d, c] = q[d]*( (Z @ WoA_aug)[d,c]/S[d] + cBf[c] ),  q = exp(bd)
     where WoA_aug = [Wo*A | 0], cBf = [bv@(Wo*A) + bo*A + shift | 1],
     A = 1+scale. Column 128 of Maug is then exactly q.
  Phase 2: T2 = expdp_chunk @ Maug ; out = edge + T2[:, :128]/T2[:, 128]
     (sum_d scores == 1 makes the constant fold exact).
"""

import math
import os

import numpy as np

from concourse import bass, mybir, tile
from concourse.bass_utils import run_bass_kernel_spmd
from concourse.masks import make_identity
from concourse.vector_clock import ScopedClock

N_CORES = 8
B, N, C, D, H = 2, 384, 128, 32, 128
M = N * N                  # 147456 edges per batch
SHARDS_PER_B = N_CORES // B
SH = M // SHARDS_PER_B     # 36864 edges per core
CHUNK = 128
NCHUNK = SH // CHUNK       # 288
SUPER = 16                 # chunks per load DMA (1 MiB)
NSUPER = NCHUNK // SUPER   # 18
EGRP = 4                   # chunks per psum/exp batch
OGRP = 4                   # chunks per output-store DMA

F32 = mybir.dt.float32
BF16 = mybir.dt.bfloat16
# matmul-operand dtype: bf16 by default (f32 accumulation everywhere, exact
# f32 residual); KERNEL_F32=1 switches the whole PE path to fp32.
MMDT = F32 if os.environ.get("KERNEL_F32", "0") == "1" else BF16
AF = mybir.ActivationFunctionType
ALU = mybir.AluOpType


# ---------------------------------------------------------------------------
# This walrus build rejects >1 sem wait on a single instruction
# ("Too many sync wait commands"); split extras onto same-engine nops.
def _patched_drain_and_barrier(self, tick_clock, wait_clock):
    probe = self.nc.sync.nop(nofuse=True)
    wait_clock.add_sem_waits(probe.ins, ScopedClock({None: tick_clock.global_clock}))
    si = probe.ins.sync_info
    waits = list(si.on_wait) if si is not None and si.on_wait else []
    if len(waits) > 1:
        si.on_wait = waits[:1]
        for w in waits[1:]:
            extra = self.nc.sync.nop(nofuse=True)
            extra.ins.sync_info = mybir.SyncInfo(on_wait=[w], on_update=[])
    self.nc.sync.drain()
    self.nc.all_engine_barrier()
    assert self.sems is not None
    popped = self.nc._tile_sem_poison_stack.pop()
    assert popped is self._sem_poison
    self.nc.clear_and_free_semaphores(list(self.sems.allocated().values()))
    self.nc.all_engine_barrier()


tile.TileContext._drain_and_barrier = _patched_drain_and_barrier

_MAXW = 1


def _split_waits(nc):
    for fn in nc.m.functions:
        for bb in fn.blocks:
            out, changed = [], False
            for inst in bb.instructions:
                si = inst.sync_info
                waits = list(si.on_wait) if si is not None and si.on_wait else []
                if len(waits) > _MAXW:
                    changed = True
                    for i, w in enumerate(waits[: -_MAXW]):
                        nop = mybir.InstNoOp(name=f"{inst.name}-sw{i}", ins=[], outs=[])
                        nop.engine = inst.engine
                        nop.sync_info = mybir.SyncInfo(on_wait=[w], on_update=[])
                        out.append(nop)
                    si.on_wait = waits[-_MAXW:]
                out.append(inst)
            if changed:
                bb.instructions = out


# ---------------------------------------------------------------------------
def _build(fast_bd0=True):
    nc = bass.Bass(
        "TRN2", target_bir_lowering=False, debug=False, num_devices=N_CORES
    )
    edge = nc.dram_tensor("edge", [SH, C], F32, kind="ExternalInput").ap()
    wcat = nc.dram_tensor("wcat", [C, 3 * D], MMDT, kind="ExternalInput").ap()
    woa = nc.dram_tensor("woa", [D, C + 1], F32, kind="ExternalInput").ap()
    cbf = nc.dram_tensor("cbf", [1, C + 1], F32, kind="ExternalInput").ap()
    qrow = nc.dram_tensor("qrow", [1, D], F32, kind="ExternalInput").ap()
    qcol = nc.dram_tensor("qcol", [D, 1], F32, kind="ExternalInput").ap()
    stk4 = nc.dram_tensor("stk4", [D, 4 * D], F32, kind="ExternalInput").ap()
    out = nc.dram_tensor("out", [SH, C], F32, kind="ExternalOutput").ap()

    edge_t = edge.rearrange("(n p) c -> p n c", p=CHUNK)     # [128, 288, 128]
    out_t = out.rearrange("(n p) c -> p n c", p=CHUNK)  # noqa: F841
    out_str = out.rearrange("(n j p) c -> p n j c", p=CHUNK, j=4)

    groups = [
        [b * SHARDS_PER_B + i for i in range(SHARDS_PER_B)] for b in range(B)
    ]

    with tile.TileContext(nc) as tc:
        with (
            tc.tile_pool(name="const", bufs=1) as const,
            tc.tile_pool(name="resident", bufs=1) as res,
            tc.tile_pool(name="dram", bufs=1, space="DRAM") as dram,
            tc.tile_pool(name="zsps", bufs=1, space="PSUM") as zsps,
        ):
            ident = const.tile([CHUNK, CHUNK], F32)
            make_identity(nc, ident)
            ident_m = ident
            if MMDT != F32:
                ident_m = const.tile([CHUNK, CHUNK], MMDT, tag="ident_m")
                make_identity(nc, ident_m)
            wcat_s = const.tile([C, 3 * D], MMDT)
            nc.sync.dma_start(out=wcat_s[:], in_=wcat[:])
            woa_s = const.tile([D, C + 1], F32)
            nc.sync.dma_start(out=woa_s[:], in_=woa[:])
            cbf_s = const.tile([1, C + 1], F32)
            nc.sync.dma_start(out=cbf_s[:], in_=cbf[:])
            qrow_s = const.tile([1, D], F32)
            nc.sync.dma_start(out=qrow_s[:], in_=qrow[:])
            qcol_s = const.tile([D, 1], F32)
            nc.sync.dma_start(out=qcol_s[:], in_=qcol[:])
            stk4_s = const.tile([D, 4 * D], F32)
            nc.sync.dma_start(out=stk4_s[:], in_=stk4[:])

            peR = res.tile([CHUNK, NCHUNK, C], F32)      # edges, resident
            eqW = res.tile([CHUNK, NCHUNK, 2 * D], MMDT)  # exp(fP)|exp(dP)
            maug_s = res.tile([4 * D, C + 1], MMDT)
            rR = None
            if fast_bd0:
                rR = res.tile([CHUNK, NCHUNK], F32, tag="rR")

            zs_ps = zsps.tile([D, D + 1], F32)

            # ---------------- Phase 1: projections + global-softmax stats
            with (
                tc.tile_pool(name="pts", bufs=3) as pts,
                tc.tile_pool(name="gbuf", bufs=4) as gbuf,
                tc.tile_pool(name="ptps", bufs=3, space="PSUM") as ptps,
                tc.tile_pool(name="pjps", bufs=3, space="PSUM") as pjps,
            ):
                done = 0
                for s in range(NSUPER):
                    nc.sync.dma_start(
                        out=peR[:, s * SUPER : (s + 1) * SUPER, :],
                        in_=edge_t[:, s * SUPER : (s + 1) * SUPER, :],
                    )
                    for q in range(SUPER // EGRP):
                        pj = pjps.tile([CHUNK, EGRP, 3 * D], F32)
                        pt_ps = ptps.tile([CHUNK, EGRP, CHUNK], F32)
                        for k in range(EGRP):
                            ck = s * SUPER + q * EGRP + k
                            nc.tensor.matmul(
                                pt_ps[:, k, :], lhsT=peR[:, ck, :],
                                rhs=ident[:], is_transpose=True,
                                start=(k == 0), stop=(k == EGRP - 1),
                            )
                        pt_s = pts.tile([CHUNK, EGRP, CHUNK], MMDT)
                        if q % 3 != 2:
                            nc.scalar.activation(pt_s[:], pt_ps[:], AF.Copy)
                        else:
                            nc.vector.tensor_copy(pt_s[:], pt_ps[:])
                        for k in range(EGRP):
                            nc.tensor.matmul(
                                pj[:, k, :], lhsT=pt_s[:, k, :], rhs=wcat_s[:],
                                start=(k == 0), stop=(k == EGRP - 1),
                            )
                        cks = s * SUPER + q * EGRP
                        nc.scalar.activation(
                            eqW[:, cks : cks + EGRP, :], pj[:, :, 0 : 2 * D],
                            AF.Exp,
                        )
                        if fast_bd0:
                            u4 = gbuf.tile([CHUNK, EGRP], F32, tag="u4")
                            nc.vector.tensor_reduce(
                                u4[:], eqW[:, cks : cks + EGRP, D : 2 * D],
                                axis=mybir.AxisListType.X, op=ALU.add,
                            )
                            nc.vector.reciprocal(
                                rR[:, cks : cks + EGRP], u4[:]
                            )
                        ga = gbuf.tile([CHUNK, EGRP, D + 1], MMDT)
                        nc.vector.tensor_copy(
                            ga[:, :, 0:D], pj[:, :, 2 * D : 3 * D]
                        )
                        nc.gpsimd.memset(ga[:, :, D : D + 1], 1.0)
                        for k in range(EGRP):
                            nc.tensor.matmul(
                                zs_ps[:],
                                lhsT=eqW[:, cks + k, 0:D],
                                rhs=ga[:, k, :],
                                start=(done == 0), stop=(done == NCHUNK - 1),
                            )
                            done += 1

            # ---------------- AllReduce of ZS within each batch group.
            # Phase-2 pools open BEFORE msps so their psum banks don't alias
            # the Maug chain — lets the exp(dP) transposes overlap the
            # collective's latency.
            with (
                tc.tile_pool(name="ets", bufs=6) as ets,
                tc.tile_pool(name="rbuf", bufs=12) as rbuf,
                tc.tile_pool(name="ostg", bufs=6) as ostg,
                tc.tile_pool(name="scl", bufs=4) as scl,
                tc.tile_pool(name="etps", bufs=3, space="PSUM") as etps,
            ):
              with (
                tc.tile_pool(name="stats", bufs=1) as stats,
                tc.tile_pool(name="msps", bufs=1, space="PSUM") as msps,
              ):
                zs_s = stats.tile([D, D + 1], F32)
                nc.vector.tensor_copy(zs_s[:], zs_ps[:])
                cc_in = dram.tile([D, D + 1], F32)
                cc_out = dram.tile([D, D + 1], F32)
                nc.sync.dma_start(out=cc_in[:], in_=zs_s[:])
                nc.gpsimd.collective_compute(
                    "AllReduce",
                    ALU.add,
                    replica_groups=groups,
                    ins=[cc_in[:].opt()],
                    outs=[cc_out[:].opt()],
                )
                zsr = stats.tile([D, D + 1], F32)
                nc.sync.dma_start(out=zsr[:], in_=cc_out[:])
                zt_ps = msps.tile([4 * D, C + 1], F32, tag="ms")
                nc.tensor.matmul(
                    zt_ps[0:D, 0:D], lhsT=zsr[:, 0:D], rhs=ident[0:D, 0:D],
                    is_transpose=True, start=True, stop=True,
                )
                zt = stats.tile([D, D], F32)
                nc.vector.tensor_copy(zt[:], zt_ps[0:D, 0:D])

                # Maug = q * (Z @ WoA_aug / S + cBf)   [32, 129]
                rs = stats.tile([D, 1], F32)
                nc.vector.reciprocal(rs[:], zsr[:, D : D + 1])
                # maug32 = rsq*(Z@WoA_aug) + q x cBf, with rsq = q/S;
                # msps tiles share one slot (sequential use) to keep the
                # phase at a single psum bank.
                qb_ps = msps.tile([4 * D, C + 1], F32, tag="ms")
                nc.tensor.matmul(
                    qb_ps[0:D, :], lhsT=qrow_s[:], rhs=cbf_s[:],
                    start=True, stop=True,
                )
                rsq = stats.tile([D, 1], F32)
                nc.vector.tensor_mul(rsq[:], rs[:], qcol_s[:])
                qb_s = stats.tile([D, C + 1], F32)
                nc.vector.tensor_copy(qb_s[:], qb_ps[0:D, :])
                zw_ps = msps.tile([4 * D, C + 1], F32, tag="ms")
                nc.tensor.matmul(
                    zw_ps[0:D, :], lhsT=zt[:], rhs=woa_s[:],
                    start=True, stop=True,
                )
                m32 = stats.tile([D, C + 1], F32)
                nc.vector.scalar_tensor_tensor(
                    out=m32[:], in0=zw_ps[0:D, :], scalar=rsq[:], in1=qb_s[:],
                    op0=ALU.mult, op1=ALU.add,
                )
                mg_ps = msps.tile([4 * D, C + 1], F32, tag="ms")
                nc.tensor.matmul(
                    mg_ps[:], lhsT=stk4_s[:], rhs=m32[:], start=True, stop=True
                )
                nc.vector.tensor_copy(maug_s[:], mg_ps[:])

              with tc.tile_pool(name="t2ps", bufs=4, space="PSUM") as t2ps:
                    # ------------ Phase 2: redistribute + FiLM + residual.
                    # Rounds of 12 chunks; psum groups hold 3 chunks that share
                    # one tile_position (HW hangs if an accumulation group mixes
                    # tile positions in one bank).
                    R12 = 12
                    for g in range(NCHUNK // R12):
                        base = g * R12
                        et_list = []
                        for t in range(3):  # 3 transpose groups of 4 chunks
                            g4 = base // 4 + t
                            et_ps = etps.tile([CHUNK, CHUNK], MMDT)
                            for kk in range(4):
                                nc.tensor.matmul(
                                    et_ps[32 * kk : 32 * (kk + 1), :],
                                    lhsT=eqW[:, g4 * 4 + kk, D : 2 * D],
                                    rhs=ident_m[:], is_transpose=True,
                                    start=(kk == 0), stop=(kk == 3),
                                    tile_position=(0, 32 * kk),
                                )
                            et_s = ets.tile([CHUNK, CHUNK], MMDT)
                            nc.vector.tensor_copy(et_s[:], et_ps[:])
                            et_list.append(et_s)
                        for j in range(4):
                            ot = ostg.tile([CHUNK, 3, C], F32)
                            t2g = t2ps.tile([CHUNK, 3, C + 1], F32)
                            for t in range(3):
                                nc.tensor.matmul(
                                    t2g[:, t, :],
                                    lhsT=et_list[t][32 * j : 32 * (j + 1), :],
                                    rhs=maug_s[32 * j : 32 * (j + 1), :],
                                    start=(t == 0), stop=(t == 2),
                                    tile_position=(32 * j, 0),
                                )
                            if fast_bd0:
                                r_t = None
                            else:
                                r_t = rbuf.tile([CHUNK, 3], F32)
                                nc.vector.reciprocal(r_t[:], t2g[:, :, C])
                            for t in range(3):
                                ck = base + 4 * t + j
                                rsc = (
                                    rR[:, ck : ck + 1] if fast_bd0
                                    else r_t[:, t : t + 1]
                                )
                                if j >= 2:  # offload half to ACT+GPSIMD
                                    sc = scl.tile([CHUNK, C], F32)
                                    nc.scalar.activation(
                                        sc[:], t2g[:, t, 0:C], AF.Copy,
                                        scale=rsc,
                                    )
                                    nc.gpsimd.tensor_tensor(
                                        out=ot[:, t, :], in0=sc[:],
                                        in1=peR[:, ck, :], op=ALU.add,
                                    )
                                else:
                                    nc.vector.scalar_tensor_tensor(
                                        out=ot[:, t, :],
                                        in0=t2g[:, t, 0:C],
                                        scalar=rsc,
                                        in1=peR[:, ck, :],
                                        op0=ALU.mult, op1=ALU.add,
                                    )
                            nc.scalar.dma_start(
                                out=out_str[
                                    :, base // 4 : base // 4 + 3, j, :
                                ],
                                in_=ot[:],
                            )
    _split_waits(nc)
    return nc


# ---------------------------------------------------------------------------
# Host side
def _gelu_exact(x):
    erf = np.vectorize(math.erf)
    return 0.5 * x * (1.0 + erf(x / math.sqrt(2.0)))


_programs = {}
LAST_PROFILE = {}


def _run(nc, in_maps, tag):
    trace = bool(int(os.environ.get("KTRACE", "0")))
    kw = {}
    if trace:
        tdir = os.path.join(os.environ.get("KTRACE_DIR", "/tmp/ktrace"), tag)
        os.makedirs(tdir, exist_ok=True)
        kw = dict(trace=True, tmpdir=tdir)
    res = run_bass_kernel_spmd(nc, in_maps, list(range(N_CORES)), **kw)
    LAST_PROFILE[tag] = res
    return res


def kernel(
    edge_embed, r3_t, Wg, bg, Wv, bv, Wd, bd, Wo, bo, Wt1, bt1, Wt2, bt2
):
    f = lambda a: np.ascontiguousarray(np.asarray(a, dtype=np.float32))
    edge_embed, r3_t = f(edge_embed), f(r3_t)
    Wg, bg, Wv, bv, Wd, bd = f(Wg), f(bg), f(Wv), f(bv), f(Wd), f(bd)
    Wo, bo, Wt1, bt1, Wt2, bt2 = f(Wo), f(bo), f(Wt1), f(bt1), f(Wt2), f(bt2)

    fast = bool(np.all(bd == 0.0))
    key = "fast" if fast else "slow"
    if key not in _programs:
        _programs[key] = _build(fast_bd0=fast)
    nc = _programs[key]

    P_all = edge_embed.reshape(B, M, C)
    shards = []
    for b in range(B):
        for i in range(SHARDS_PER_B):
            shards.append(P_all[b, i * SH : (i + 1) * SH])

    # bg drops out of softmax over m; bv/bd/bo/FiLM folded below.
    import ml_dtypes
    mmdt_np = np.float32 if MMDT == F32 else ml_dtypes.bfloat16
    wcat = np.ascontiguousarray(
        np.concatenate([Wg, Wd, Wv], axis=1).astype(mmdt_np)
    )

    t_emb = _gelu_exact(r3_t.reshape(B, 1) @ Wt1 + bt1).astype(np.float32)
    t_emb = (t_emb @ Wt2 + bt2).astype(np.float32)
    scale, shift = t_emb[:, :C], t_emb[:, C:]
    q = np.exp(bd).astype(np.float32)
    stk4 = np.ascontiguousarray(np.tile(np.eye(D, dtype=np.float32), (1, 4)))

    in_maps = []
    for ci in range(N_CORES):
        b = ci // SHARDS_PER_B
        A = 1.0 + scale[b]
        Bf = bo * A + shift[b]
        WoA = Wo * A[None, :]
        woa_aug = np.concatenate([WoA, np.zeros((D, 1), np.float32)], axis=1)
        cbf = np.concatenate([bv @ WoA + Bf, np.ones((1,), np.float32)])
        in_maps.append(
            {
                "edge": shards[ci],
                "wcat": wcat,
                "woa": np.ascontiguousarray(woa_aug),
                "cbf": np.ascontiguousarray(cbf.reshape(1, C + 1)),
                "qrow": np.ascontiguousarray(q.reshape(1, D)),
                "qcol": np.ascontiguousarray(q.reshape(D, 1)),
                "stk4": stk4,
            }
        )

    res = _run(nc, in_maps, "main")

    out = np.empty((B, M, C), np.float32)
    for b in range(B):
        for i in range(SHARDS_PER_B):
            out[b, i * SH : (i + 1) * SH] = res.results[b * SHARDS_PER_B + i][
                "out"
            ]
    return out.reshape(B, N, N, C)

